# revision 1
# baseline (speedup 1.0000x reference)
"""Trainium2 Bass kernel for the DenoisingModule (non-local attention block).

Math (see reference):
    theta = Wt @ x + bt            [B, 128, HW]
    phi   = Wp @ x + bp            [B, 128, HW]
    f     = theta^T @ phi / 16     [B, HW, HW]
    fh    = softmax(f, axis=0)     (over the BATCH axis - PyTorch legacy dim=0)
    den   = fh @ x^T               [B, C, HW]
    out   = den + (Wc @ den + bc)  = (I + Wc) @ den + bc

Sharding: the softmax couples all 8 batch elements at each (n, m) position,
so batch-parallel would need a 64MB cross-device all-reduce.  Instead we
shard the *n* axis (rows of f / output pixels): each of the 8 cores owns
n in [k*512, (k+1)*512), holds full x, and the softmax is fully local.
No collectives at all; host slices inputs and concatenates outputs.

Per-core pipeline (m streamed in 8 chunks of 512):
    theta_local (once)  : PE, K-split 256->2, scaled by 1/16 host-side (exact)
    phi chunk           : PE from streamed x tiles
    f~ tiles [m128,n512]: PE, lhsT=phi slice, rhs=theta_local  (float32r)
    fexp = exp(f~)      : ScalarE, PSUM->SBUF
    S = sum_b fexp      : VectorE chain adds; R = 1/S (reciprocal_approx_fast)
    fh = fexp * R       : VectorE, in-place
    den += x^T-tile @ fh: PE accumulate in PSUM, VectorE spill-add to SBUF
    out = (I+Wc) @ den  : PE + ScalarE bias copy, DMA out

Matmuls use float32r (4-byte fp32 data, PE fast mode ~1 col/cycle vs 4
for plain fp32; measured output rel err ~2e-4); everything else fp32.
1/S is computed as exp(-ln S) on ScalarE (the custom-DVE fast reciprocal
trips this walrus build, and Exp/Ln share one activation table set).

The installed walrus rejects any engine/DMA instruction carrying more
than one semaphore wait ("Too many sync wait commands"), but Tile's
sem-assignment emits up to 4.  _split_excess_waits() legalizes the
scheduled program post-hoc by hoisting excess waits onto single-wait
EventSemaphore instructions inserted just before, on the same engine
queue (applied on the hardware path only; CoreSim runs the pre-split
program).
"""

import sys

import numpy as np

B = 8
C = 256
D = C // 2  # 128
HW = 4096
NCORES = 8
NLOC = HW // NCORES  # 512 n-columns per core
MC = 512  # m-chunk size
NCHUNK = HW // MC  # 8
P = 128

TRACE = False
LAST = {}

# pool-size knobs (TimelineSim-tuned)
XN_BUFS = 6
XT_BUFS = 24
PSA_BUFS = 2
PSD_BUFS = 2
SMX_BUFS = 1
OUT_BUFS = 2
FEXP2 = 0  # number of batch indices whose fexp tile is double-buffered

_prog = None


def _ensure_path():
    try:
        import concourse  # noqa: F401
    except ImportError:
        for p in ("/opt/trn_rl_repo", "/root/.axon_site/_ro/trn_rl_repo"):
            if p not in sys.path:
                sys.path.insert(0, p)
        import concourse  # noqa: F401


def _build():
    from contextlib import ExitStack

    import concourse.bass as bass
    import concourse.tile as tile
    from concourse import mybir

    f32 = mybir.dt.float32
    f32r = mybir.dt.float32r
    AF = mybir.ActivationFunctionType

    nc = bass.Bass(trn_type="TRN2", target_bir_lowering=False, debug=False)

    xs_h = nc.dram_tensor("xs", [B, C, NLOC], f32r, kind="ExternalInput")
    xn_h = nc.dram_tensor("xn", [B, C, HW], f32r, kind="ExternalInput")
    xt_h = nc.dram_tensor("xt", [B, HW, C], f32r, kind="ExternalInput")
    wthT_h = nc.dram_tensor("wthT", [C, D], f32r, kind="ExternalInput")
    wphT_h = nc.dram_tensor("wphT", [C, D], f32r, kind="ExternalInput")
    wcT_h = nc.dram_tensor("wcT", [C, C], f32, kind="ExternalInput")
    bth_h = nc.dram_tensor("bth", [D, 1], f32, kind="ExternalInput")
    bph_h = nc.dram_tensor("bph", [D, 1], f32, kind="ExternalInput")
    bc_h = nc.dram_tensor("bc", [C, 1], f32, kind="ExternalInput")
    out_h = nc.dram_tensor("out", [B, C, NLOC], f32, kind="ExternalOutput")

    xs = xs_h.ap()
    xn = xn_h.ap()
    xt = xt_h.ap()
    out = out_h.ap()

    def asf32(ap):
        return ap.bitcast(f32)

    with tile.TileContext(nc) as tc:
        with ExitStack() as ctx:
            consts = ctx.enter_context(tc.tile_pool(name="consts", bufs=1))
            theta_p = ctx.enter_context(tc.tile_pool(name="theta", bufs=1))
            xs_p = ctx.enter_context(tc.tile_pool(name="xsp", bufs=2))
            xn_p = ctx.enter_context(tc.tile_pool(name="xnp", bufs=XN_BUFS))
            xt_p = ctx.enter_context(tc.tile_pool(name="xtp", bufs=XT_BUFS))
            phi_p = ctx.enter_context(tc.tile_pool(name="phip", bufs=1))
            fexp_p = ctx.enter_context(tc.tile_pool(name="fexpp", bufs=1))
            smx_p = ctx.enter_context(tc.tile_pool(name="smxp", bufs=SMX_BUFS))
            den_p = ctx.enter_context(tc.tile_pool(name="denp", bufs=1))
            out_p = ctx.enter_context(tc.tile_pool(name="outp", bufs=OUT_BUFS))
            psA = ctx.enter_context(tc.tile_pool(name="psA", bufs=PSA_BUFS, space="PSUM"))
            psD = ctx.enter_context(tc.tile_pool(name="psD", bufs=PSD_BUFS, space="PSUM"))

            # ---- constants ----
            wth_sb = []
            wph_sb = []
            wc_sb = []
            for ck in range(2):
                t = consts.tile([P, D], f32r, name=f"wth{ck}", tag=f"wth{ck}")
                nc.sync.dma_start(out=t, in_=wthT_h.ap()[ck * P:(ck + 1) * P, :])
                wth_sb.append(t)
                t = consts.tile([P, D], f32r, name=f"wph{ck}", tag=f"wph{ck}")
                nc.sync.dma_start(out=t, in_=wphT_h.ap()[ck * P:(ck + 1) * P, :])
                wph_sb.append(t)
                t = consts.tile([P, C], f32, name=f"wc{ck}", tag=f"wc{ck}")
                nc.sync.dma_start(out=t, in_=wcT_h.ap()[ck * P:(ck + 1) * P, :])
                wc_sb.append(t)
            bth_sb = consts.tile([D, 1], f32, name="bth", tag="bth")
            nc.sync.dma_start(out=bth_sb, in_=bth_h.ap()[:, :])
            bph_sb = consts.tile([D, 1], f32, name="bph", tag="bph")
            nc.sync.dma_start(out=bph_sb, in_=bph_h.ap()[:, :])
            bc_sb = []
            for dk in range(2):
                t = consts.tile([P, 1], f32, name=f"bc{dk}", tag=f"bc{dk}")
                nc.sync.dma_start(out=t, in_=bc_h.ap()[dk * P:(dk + 1) * P, :])
                bc_sb.append(t)

            # ---- theta_local: [d=128, n=512] per batch, scaled 1/16 ----
            theta_sb = []
            for b in range(B):
                ps = psA.tile([P, 2 * NLOC], f32, name=f"psth{b}", tag="psA")
                xst_l = []
                for ck in range(2):
                    xst = xs_p.tile([P, NLOC], f32r, name=f"xs{b}_{ck}", tag="xs")
                    nc.sync.dma_start(out=xst, in_=xs[b, ck * P:(ck + 1) * P, :])
                    xst_l.append(xst)
                for ck in range(2):
                    nc.tensor.matmul(
                        ps[:, :NLOC], wth_sb[ck], xst_l[ck],
                        start=(ck == 0), stop=(ck == 1),
                    )
                th = theta_p.tile([D, NLOC], f32r, name=f"theta{b}", tag=f"theta{b}")
                nc.scalar.activation(th, ps[:, :NLOC], AF.Identity, bias=bth_sb)
                theta_sb.append(th)

            # ---- main loop over m-chunks ----
            den_sb = [None] * B
            for mc in range(NCHUNK):
                m0 = mc * MC
                # phi for this chunk: [d=128, m=512] per batch
                phi_sb = []
                for b in range(B):
                    ps = psA.tile([P, 2 * NLOC], f32, name=f"psph{mc}_{b}", tag="psA")
                    xnt_l = []
                    for ck in range(2):
                        xnt = xn_p.tile([P, MC], f32r, name=f"xn{mc}_{b}_{ck}",
                                        tag="xn")
                        nc.sync.dma_start(
                            out=xnt, in_=xn[b, ck * P:(ck + 1) * P, m0:m0 + MC],
                        )
                        xnt_l.append(xnt)
                    for ck in range(2):
                        nc.tensor.matmul(
                            ps[:, :MC], wph_sb[ck], xnt_l[ck],
                            start=(ck == 0), stop=(ck == 1),
                        )
                    ph = phi_p.tile([D, MC], f32r, name=f"phi{mc}_{b}", tag=f"phi{b}")
                    nc.scalar.activation(ph, ps[:, :MC], AF.Identity, bias=bph_sb)
                    phi_sb.append(ph)

                # xT tiles for the den matmuls (prefetch early)
                xt_t = [[None] * 4 for _ in range(B)]
                for b in range(B):
                    for s in range(4):
                        t = xt_p.tile([P, C], f32r, name=f"xt{mc}_{b}_{s}", tag="xt")
                        nc.sync.dma_start(
                            out=t, in_=xt[b, m0 + s * P:m0 + (s + 1) * P, :],
                        )
                        xt_t[b][s] = t

                # f~ = theta'^T phi, exp -> fexp [m=128, (s,n) free]
                fexp = []
                for b in range(B):
                    fe = fexp_p.tile([P, 4 * NLOC], f32r, name=f"fexp{mc}_{b}",
                                     tag=f"fexp{b}", bufs=(2 if b < FEXP2 else 1))
                    for sp in range(2):
                        ps = psA.tile([P, 2 * NLOC], f32,
                                      name=f"psf{mc}_{b}_{sp}", tag="psA")
                        for si in range(2):
                            s = sp * 2 + si
                            nc.tensor.matmul(
                                ps[:, si * NLOC:(si + 1) * NLOC],
                                phi_sb[b][:, s * P:(s + 1) * P],
                                theta_sb[b],
                                start=True, stop=True,
                            )
                        nc.scalar.activation(
                            fe[:, sp * 2 * NLOC:(sp + 1) * 2 * NLOC], ps, AF.Exp
                        )
                    fexp.append(fe)

                # softmax over batch: S = sum_b fexp[b]; R = 1/S; fh = fexp*R
                S = smx_p.tile([P, 4 * NLOC], f32, name=f"S{mc}", tag="S")
                nc.vector.tensor_add(S, asf32(fexp[0]), asf32(fexp[1]))
                for b in range(2, B):
                    nc.vector.tensor_add(S, S, asf32(fexp[b]))
                # R = 1/S computed as exp(-ln S): two ScalarE ops (same
                # activation table set as the main exp), avoids the slow
                # DVE iterative reciprocal and custom-ISA ops.
                lnS = smx_p.tile([P, 4 * NLOC], f32, name=f"lnS{mc}", tag="lnS")
                nc.scalar.activation(lnS, S, AF.Ln)
                R = smx_p.tile([P, 4 * NLOC], f32, name=f"R{mc}", tag="R")
                nc.scalar.activation(R, lnS, AF.Exp, scale=-1.0)
                for b in range(B):
                    nc.vector.tensor_mul(fexp[b], asf32(fexp[b]), R)

                # den[b] += xT^T @ fh   ([c=256 over 2 banks, n=512])
                for b in range(B):
                    psd = psD.tile([P, 2 * NLOC], f32, name=f"psd{mc}_{b}", tag="psD")
                    for ct in range(2):
                        for s in range(4):
                            nc.tensor.matmul(
                                psd[:, ct * NLOC:(ct + 1) * NLOC],
                                xt_t[b][s][:, ct * P:(ct + 1) * P],
                                fexp[b][:, s * NLOC:(s + 1) * NLOC],
                                start=(s == 0), stop=(s == 3),
                            )
                    if mc == 0:
                        dn = den_p.tile([P, 2 * NLOC], f32, name=f"den{b}",
                                        tag=f"den{b}")
                        nc.vector.tensor_copy(dn, psd)
                        den_sb[b] = dn
                    else:
                        nc.vector.tensor_add(den_sb[b], den_sb[b], psd)

            # ---- out = (I + Wc) @ den + bc ---- (plain fp32 matmul)
            for b in range(B):
                for dk in range(2):
                    ps = psA.tile([P, 2 * NLOC], f32, name=f"pso{b}_{dk}", tag="psA")
                    for ct in range(2):
                        nc.tensor.matmul(
                            ps[:, :NLOC],
                            wc_sb[ct][:, dk * P:(dk + 1) * P],
                            den_sb[b][:, ct * NLOC:(ct + 1) * NLOC],
                            start=(ct == 0), stop=(ct == 1),
                        )
                    ot = out_p.tile([P, NLOC], f32, name=f"out{b}_{dk}", tag="out")
                    nc.scalar.activation(ot, ps[:, :NLOC], AF.Identity,
                                         bias=bc_sb[dk])
                    nc.sync.dma_start(out=out[b, dk * P:(dk + 1) * P, :], in_=ot)

    return nc


def _split_excess_waits(nc, mybir, cap=1):
    """The installed walrus rejects engine instructions carrying more than
    one semaphore wait (setupSyncWait: "Too many sync wait commands"), but
    Tile's sem-assignment emits up to 4.  Legalize post-hoc: merge same-sem
    waits (max value), keep one on the instruction, and hoist the rest onto
    single-wait EventSemaphore instructions inserted just before, on the
    same engine queue (applies to every opcode incl. DMA pseudo-ops)."""
    n_ev = 0
    for fn in nc.m.functions:
        for blk in fn.blocks:
            insts = blk.instructions
            out = []
            changed = False
            for i in insts:
                si = getattr(i, "sync_info", None)
                waits = list(si.on_wait) if si is not None and si.on_wait else []
                if len(waits) > 1:
                    merged = {}
                    for w in waits:
                        k = w.id
                        if k not in merged or merged[k].wait_value < w.wait_value:
                            merged[k] = w
                    waits = list(merged.values())
                    while len(waits) > cap:
                        w = waits.pop(0)
                        ev = mybir.InstEventSemaphore(
                            name=f"{i.name}-wsplit{n_ev}", engine=i.engine)
                        ev.sync_info = mybir.SyncInfo(on_wait=[w], on_update=[])
                        try:
                            ev.debug = i.debug
                        except Exception:
                            pass
                        out.append(ev)
                        n_ev += 1
                    si.on_wait = waits
                    changed = True
                out.append(i)
            if changed:
                blk.instructions = out
    return n_ev


def _host_prep(x, w_theta, b_theta, w_phi, b_phi, w_conv, b_conv):
    x = np.asarray(x, dtype=np.float32)
    w_theta = np.asarray(w_theta, dtype=np.float32)
    b_theta = np.asarray(b_theta, dtype=np.float32)
    w_phi = np.asarray(w_phi, dtype=np.float32)
    b_phi = np.asarray(b_phi, dtype=np.float32)
    w_conv = np.asarray(w_conv, dtype=np.float32)
    b_conv = np.asarray(b_conv, dtype=np.float32)

    xr = np.ascontiguousarray(x.reshape(B, C, HW))
    xtr = np.ascontiguousarray(xr.transpose(0, 2, 1))
    # 1/sqrt(C) = 1/16: exact power-of-two scale folded into theta
    wthT = np.ascontiguousarray((w_theta * (1.0 / 16.0)).T)
    wphT = np.ascontiguousarray(w_phi.T)
    wcT = np.ascontiguousarray((np.eye(C, dtype=np.float32) + w_conv).T)
    bth = np.ascontiguousarray((b_theta * (1.0 / 16.0)).reshape(D, 1))
    bph = np.ascontiguousarray(b_phi.reshape(D, 1))
    bc = np.ascontiguousarray(b_conv.reshape(C, 1))

    in_maps = []
    for k in range(NCORES):
        xs_k = np.ascontiguousarray(xr[:, :, k * NLOC:(k + 1) * NLOC])
        in_maps.append({
            "xs": xs_k, "xn": xr, "xt": xtr,
            "wthT": wthT, "wphT": wphT, "wcT": wcT,
            "bth": bth, "bph": bph, "bc": bc,
        })
    return in_maps


def kernel(x, w_theta, b_theta, w_phi, b_phi, w_conv, b_conv):
    global _prog
    _ensure_path()
    from concourse.bass_utils import run_bass_kernel_spmd

    if _prog is None:
        _prog = _build()
        from concourse import mybir
        _split_excess_waits(_prog, mybir)

    in_maps = _host_prep(x, w_theta, b_theta, w_phi, b_phi, w_conv, b_conv)
    res = run_bass_kernel_spmd(
        _prog, in_maps, list(range(NCORES)), trace=TRACE
    )
    LAST["res"] = res

    outf = np.empty((B, C, HW), dtype=np.float32)
    for k in range(NCORES):
        outf[:, :, k * NLOC:(k + 1) * NLOC] = res.results[k]["out"]
    return outf.reshape(B, C, 64, 64)

